# revision 39
# baseline (speedup 1.0000x reference)
"""Contrastive loss (CLIP-style, 2 views) on 8 Trainium2 NeuronCores.

Math: with Af/Bf the L2-normalized (V*N, D) view-major matrices,
  loss = mean_i [ logsumexp_{j != i}(Af@Bf.T / T)[i, :] - (Af@Bf.T)[i, p(i)]/T ]
where p(i) = (i + N) mod (V*N) is the other-view partner of row i.

Sharding: rows of Af are split across 8 cores (1024 rows each); every core
gets the full B (D-major) with its columns rotated by 1024*k so the
diagonal of core k's slab lands at *static* local columns (row-chunk m ->
cols [128m, 128m+128) of column-group 0) and the partner diagonal at the
same offset of column-group 2.  SPMD program identical on all cores.

v2 design (from trace analysis of the 133 us baseline):
- Inputs ship compressed: at fp8e4 (matmul lhsT directly), arow/bt bf16.
  DMA drops 10 MB -> ~5 MB per core, prologue halves.
- ACT runs ONLY: a dummy ln (table preload), the ln/exp rsqrt chains for
  A-row norms + B group-0 column norms (all in the otherwise-idle
  prologue, 2 table loads total), then 24 of the 32 exp tiles.
- 8 exp tiles (odd-m of groups 1 and 3) run on the DVE via the Schraudolph
  int-trick: i32(x*A + B) bit-cast is exp(x) to ~1.8% sawtooth with a
  C constant tuned for zero-mean log error; a reduce pass makes row sums.
- B groups 1-3 column rsqrt: ones-matmul (fp8 DoubleRow) -> PSUM row ->
  SBUF -> DRAM bounce -> compact (128,16) -> Quake rsqrt w/ 1 Newton step
  on the GPSIMD (idle engine; 0.17% max err) -> DRAM -> partition-
  broadcast multiply.  DVE only does the row copy + final multiplies.
- B squares: group 0+1 on DVE (latency critical), 2+3 on GPSIMD.
- All norm matmuls in fp8 DoubleRow (ones fp8), halving their PE cost.
"""

import os

import numpy as np

N = 4096
V = 2
D = 256
M = V * N            # 8192 rows/cols of the logits matrix
TEMP = 0.07
NCORES = 8
ROWS = M // NCORES   # 1024 rows per core
P = 128              # partitions
NM = ROWS // P       # 8 row-chunks per core
GW = 2048            # column-group width (one B normalize unit)
NG = M // GW         # 4 column groups
PSW = 2048           # PSUM tile width (half of PSUM -> 2-deep rotation)
KC = D // P          # 2 contraction chunks
NEG = -1.0e9         # additive mask for the diagonal
MAGIC = 0x5F3759DF   # Quake rsqrt seed
SB_A = 12102203.161561485          # 2^23 / ln 2
SB_B = 1065353216.0 - 480000.0     # 127*2^23 - C, C tuned for 0-mean log err

# which (g, m) exp tiles run on the DVE instead of ACT
def _is_dve_tile(g, m):
    return (g == 1 and m in (1, 3, 5)) or (g == 3 and m in (1, 5)) \
        or (g == 2 and m in (2, 6))

_CACHE: dict = {}


def _build_nc():
    import concourse.bacc as bacc
    import concourse.bass as bass
    import concourse.mybir as mybir
    import concourse.tile as tile

    f32 = mybir.dt.float32
    i32 = mybir.dt.int32
    bf16 = mybir.dt.bfloat16
    fp8 = mybir.dt.float8e4
    AX = mybir.AxisListType
    OP = mybir.AluOpType
    AF = mybir.ActivationFunctionType
    DR = mybir.MatmulPerfMode.DoubleRow

    nc = bacc.Bacc("TRN2", target_bir_lowering=False, debug=False,
                   num_devices=NCORES)

    at_d = nc.dram_tensor("at", (P, KC, ROWS), fp8, kind="ExternalInput")
    arow_d = nc.dram_tensor("arow", (P, NM, D), bf16, kind="ExternalInput")
    bt0_d = nc.dram_tensor("bt0", (P, 4, KC, 512), bf16, kind="ExternalInput")
    btr_d = nc.dram_tensor("btr", (P, NG - 1, KC, GW), bf16,
                           kind="ExternalInput")
    i128_d = nc.dram_tensor("i128", (P, P), bf16, kind="ExternalInput")
    mneg_d = nc.dram_tensor("mneg", (P, KC, P), fp8, kind="ExternalInput")
    mpos_d = nc.dram_tensor("mpos", (P, KC, P), fp8, kind="ExternalInput")
    out_d = nc.dram_tensor("stats", (P, 2 * NM), f32, kind="ExternalOutput")

    with tile.TileContext(nc) as tc:
        with (
            tc.tile_pool(name="big", bufs=1) as big,
            tc.tile_pool(name="work", bufs=2) as work,
            tc.tile_pool(name="dram", bufs=2,
                         space=bass.MemorySpace.DRAM) as dr,
            tc.tile_pool(name="psum", bufs=2, space=bass.MemorySpace.PSUM) as pp,
        ):
            # --- persistent SBUF tensors -------------------------------
            at_b = big.tile((P, KC, ROWS), fp8)     # A slab (matmul lhsT)
            arow_s = big.tile((P, NM, D), bf16)     # A slab, row-major
            bt_b = big.tile((P, KC, M), fp8)        # normalized B (rhs)
            i128_s = big.tile((P, P), bf16)
            mneg_s = big.tile((P, KC, P), fp8)      # -16 I (diag mask mm)
            mpos_s = big.tile((P, KC, P), fp8)      # +16 I
            ones_b = big.tile((P, KC, P), fp8)      # ones (norm mm)
            ssa_s = big.tile((P, NM), f32)          # sum(a^2) per slab row
            lssa_s = big.tile((P, NM), f32)         # ln of it
            sca_s = big.tile((P, NM), f32)          # 1/(|a|*T) exp scales
            sca2_s = big.tile((P, NM), f32)         # * SB_A, for DVE tiles
            acc_s = big.tile((P, NM, NG), f32)      # exp row-sums per tile
            cat_s = big.tile((P, 2 * NM), f32)      # [S | exp(pos)] per row
            lns0_s = big.tile((P, GW), f32)         # ln of g0 col sumsq
            inv0_s = big.tile((P, GW), bf16)        # g0 col rsqrt
            blnt_s = big.tile((P, 1), f32)          # ln(1/T) bias for sca

            # --- Quake rsqrt, 1 Newton step (max rel err 0.175%) -------
            # shift on DVE (tensor_scalar unsupported on Pool); arithmetic
            # as tensor_tensor on GPSIMD against memset const tiles
            QSH = (16, P)
            magic_s = big.tile(QSH, i32)
            c15_s = big.tile(QSH, f32)
            cnh_s = big.tile(QSH, f32)
            onei_s = big.tile(QSH, i32)

            def quake_rsqrt(out_ap, s_ap):
                ii = work.tile(QSH, i32, tag="qk_i", bufs=3)
                t1 = work.tile(QSH, f32, tag="qk_t", bufs=3)
                nc.vector.tensor_scalar(ii[:], s_ap.bitcast(i32), 1,
                                        None, OP.logical_shift_right)
                nc.gpsimd.tensor_tensor(ii[:], magic_s[:], ii[:],
                                        OP.subtract)
                yb = ii[:].bitcast(f32)
                # y = yb * (1.5 - 0.5*s*yb^2)
                nc.gpsimd.tensor_mul(t1[:], yb, yb)
                nc.gpsimd.tensor_mul(t1[:], t1[:], s_ap)
                nc.gpsimd.tensor_mul(t1[:], t1[:], cnh_s[:])
                nc.gpsimd.tensor_tensor(t1[:], t1[:], c15_s[:], OP.add)
                nc.gpsimd.tensor_mul(out_ap, yb, t1[:])

            # --- table preload + DMA issue order -----------------------
            # pre-place the combined ln+exp table (set 6 in act_info.json)
            # so the insert_act_table_loads fixpoint sees every ln/exp
            # covered and emits nothing -> zero mid-kernel table switches
            nc.scalar.add_instruction(
                mybir.InstLoadActFuncSet(
                    name=nc.get_next_instruction_name(),
                    ins=[], outs=[], act_func_set_id=6))
            nc.scalar.dma_start(arow_s[:], arow_d.ap())
            nc.scalar.dma_start(at_b[:], at_d.ap())
            btf_tiles = []
            for _g in range(NG):
                btf = work.tile((P, KC, GW), bf16, tag="btf", bufs=4)
                btf_tiles.append(btf)

            for c in range(GW // 512):
                csl = slice(c * 512, (c + 1) * 512)
                nc.sync.dma_start(btf_tiles[0][:, :, csl], bt0_d.ap()[:, c])
            nc.sync.dma_start(mneg_s[:], mneg_d.ap())
            nc.sync.dma_start(mpos_s[:], mpos_d.ap())
            nc.sync.dma_start(i128_s[:], i128_d.ap())
            def issue_b(g):
                nc.sync.dma_start(btf_tiles[g][:, :, :], btr_d.ap()[:, g - 1])

            issue_b(1)


            nc.vector.memset(ones_b[:], 1.0)
            nc.vector.memset(blnt_s[:], float(np.log(1.0 / TEMP)))
            nc.gpsimd.memset(magic_s[:], MAGIC)
            nc.gpsimd.memset(onei_s[:], 1)
            nc.gpsimd.memset(c15_s[:], 1.5)
            nc.gpsimd.memset(cnh_s[:], -0.5)

            # --- group 0 normalization: fully chunk-pipelined ----------
            # ones-matmul broadcasts the column sumsq to all partitions;
            # rsqrt = exp(-0.5*ln) per 512-chunk on the idle ACT; chunks
            # 0-1 squared/normalized on DVE, 2-3 on GPSIMD
            btf0 = btf_tiles[0]
            ssb0 = pp.tile((P, PSW), f32, tag="ps", bufs=2)
            bsq0 = work.tile((P, KC, GW), fp8, tag="bsq0", bufs=1)
            for c in range(GW // 512):
                csl = slice(c * 512, (c + 1) * 512)
                if c < 2:
                    nc.scalar.activation(bsq0[:, :, csl], btf0[:, :, csl],
                                         AF.Square)
                else:
                    nc.vector.tensor_mul(bsq0[:, :, csl], btf0[:, :, csl],
                                         btf0[:, :, csl])
                nc.tensor.matmul(ssb0[:, csl], ones_b[:], bsq0[:, :, csl],
                                 start=True, stop=True, perf_mode=DR)
                nc.scalar.activation(lns0_s[:, csl], ssb0[:, csl], AF.Ln)
                nc.scalar.activation(inv0_s[:, csl], lns0_s[:, csl],
                                     AF.Exp, scale=-0.5)
            # A scales on ACT: sca = exp(-0.5*ln(ssa) + ln(1/T))
            for m in range(NM):
                asq = work.tile((P, D), bf16, tag="asq", bufs=2)
                nc.vector.scalar_tensor_tensor(
                    asq[:], arow_s[:, m, :], 0.0, arow_s[:, m, :],
                    OP.bypass, OP.mult,
                    accum_out=ssa_s[:, m : m + 1])
            nc.scalar.activation(lssa_s[:], ssa_s[:], AF.Ln)
            nc.scalar.activation(sca_s[:], lssa_s[:], AF.Exp,
                                 scale=-0.5, bias=blnt_s[:])
            nc.vector.tensor_scalar_mul(sca2_s[:], sca_s[:], SB_A)
            for c in range(GW // 512):
                csl = slice(c * 512, (c + 1) * 512)
                eng0 = nc.vector if c < 2 else nc.gpsimd
                for kc in range(KC):
                    eng0.tensor_mul(bt_b[:, kc, csl], btf0[:, kc, csl],
                                    inv0_s[:, csl])

            # --- B groups 1-3 norm chain pieces ------------------------
            bsq_tiles = {}

            def sq_chunk(g, c, eng0):
                if g not in bsq_tiles:
                    bsqn = work.tile((P, KC, GW), fp8, tag="bsq", bufs=2)
                    bsq_tiles[g] = bsqn
                bsq = bsq_tiles[g]
                csl = slice(c * 512, (c + 1) * 512)
                eng0.tensor_mul(bsq[:, :, csl], btf_tiles[g][:, :, csl],
                                btf_tiles[g][:, :, csl])
                return bsq

            def norm_pre(g, bsq):
                # row-0 sumsq via fp8 DR ones-matmul (chunked so the
                # borrowed PSUM slot frees fast), bf16 row copy, DRAM
                # bounce to a compact (16,128) tile
                ssb = pp.tile((P, PSW), f32, tag="ps", bufs=2)
                brow = work.tile((P, GW), bf16, tag="brow", bufs=2)
                for c in range(GW // 512):
                    csl = slice(c * 512, (c + 1) * 512)
                    nc.tensor.matmul(ssb[:, csl], ones_b[:], bsq[:, :, csl],
                                     start=True, stop=True, perf_mode=DR)
                    nc.vector.tensor_copy(brow[0:1, csl], ssb[0:1, csl])
                dsb = dr.tile((GW,), bf16, tag="dsB", bufs=3)
                nc.sync.dma_start(dsb[:], brow[0:1, :])
                compb = work.tile((16, P), bf16, tag="compb", bufs=3)
                nc.sync.dma_start(
                    compb[:], dsb[:].rearrange("(p c) -> p c", p=16))
                return compb

            def norm_quake(g, compb):
                compf = work.tile((16, P), f32, tag="compf", bufs=3)
                nc.gpsimd.tensor_copy(compf[:], compb[:])
                invb = work.tile((16, P), bf16, tag="invb", bufs=3)
                quake_rsqrt(invb[:], compf[:])
                drb = dr.tile((GW,), bf16, tag="drB", bufs=3)
                nc.sync.dma_start(
                    drb[:].rearrange("(p c) -> p c", p=16), invb[:])
                rbc = work.tile((P, GW), bf16, tag="rbc", bufs=2)
                for c in range(GW // 512):
                    csl = slice(c * 512, (c + 1) * 512)
                    nc.sync.dma_start(
                        rbc[:, csl],
                        drb[:][c * 512 : (c + 1) * 512].partition_broadcast(P))
                return rbc

            def norm_mult(g, rbc, c, eng0):
                base_g = g * GW
                csl = slice(c * 512, (c + 1) * 512)
                osl = slice(base_g + c * 512, base_g + (c + 1) * 512)
                for kc in range(KC):
                    eng0.tensor_mul(bt_b[:, kc, osl],
                                    btf_tiles[g][:, kc, csl], rbc[:, csl])

            for c in range(GW // 512):
                sq_chunk(1, c, nc.vector)

            # --- phase 1: logits + exp row-sums ------------------------
            state = {}
            for g in range(NG):
                base = g * GW
                for m in range(NM):
                    lg = pp.tile((P, PSW), f32, tag="ps", bufs=2)
                    msl = slice(m * P, (m + 1) * P)
                    for c in range(PSW // 512):
                        csl = slice(c * 512, (c + 1) * 512)
                        bsl = slice(base + c * 512, base + (c + 1) * 512)
                        masked = g == 0 and c == (m * P) // 512
                        nc.tensor.matmul(
                            lg[:, csl],
                            at_b[:, :, m * P : (m + 1) * P],
                            bt_b[:, :, bsl],
                            start=True, stop=not masked, perf_mode=DR)
                        if masked:
                            # accumulate -256 on the diagonal -> exp == 0
                            nc.tensor.matmul(
                                lg[:, msl], mneg_s[:], mpos_s[:],
                                start=False, stop=True, perf_mode=DR)
                    if _is_dve_tile(g, m):
                        # Schraudolph: exp(x) ~= bitcast(i32(x*A + B))
                        eint = work.tile((P, PSW), i32, tag="eint", bufs=3)
                        nc.vector.tensor_scalar(
                            eint[:], lg[:], sca2_s[:, m : m + 1], SB_B,
                            OP.mult, OP.add)
                        nc.vector.reduce_sum(
                            acc_s[:, m, g : g + 1], eint[:].bitcast(f32),
                            axis=AX.X)
                        if g == 2:
                            pscr = work.tile((P, P), bf16, tag="pscr",
                                             bufs=3)
                            nc.vector.scalar_tensor_tensor(
                                pscr[:], eint[:, msl].bitcast(f32), 0.0,
                                i128_s[:], OP.bypass, OP.mult,
                                accum_out=cat_s[:, NM + m : NM + m + 1])
                    else:
                        esc = work.tile((P, PSW), bf16, tag="esc", bufs=4)
                        nc.scalar.activation(
                            esc[:], lg[:], AF.Exp,
                            scale=sca_s[:, m : m + 1],
                            accum_out=acc_s[:, m, g : g + 1])
                        if g == 2:
                            # partner (positive): ln(exp diag) on host
                            pscr = work.tile((P, P), bf16, tag="pscr",
                                             bufs=3)
                            nc.vector.scalar_tensor_tensor(
                                pscr[:], esc[:, msl], 0.0, i128_s[:],
                                OP.bypass, OP.mult,
                                accum_out=cat_s[:, NM + m : NM + m + 1])
                    # norm chains: GPSIMD is the dedicated chain engine
                    # (quake + all normalize mults) so bounce latency
                    # never head-of-line blocks a sweep engine; ssb
                    # matmuls enter the PE stream early, when squares
                    # are already done
                    if g == 0:
                        if m == 0:
                            state["cb1"] = norm_pre(1, bsq_tiles[1])
                        if m == 1:
                            state["rbc1"] = norm_quake(1, state["cb1"])
                        if m == 2:
                            issue_b(2)
                            for cc in range(4):
                                norm_mult(1, state["rbc1"], cc, nc.gpsimd)
                        if m in (3, 4):
                            sq_chunk(2, 2 * (m - 3), nc.vector)
                            sq_chunk(2, 2 * (m - 3) + 1, nc.vector)
                        if m == 5:
                            state["cb2"] = norm_pre(2, bsq_tiles[2])
                        if m == 6:
                            state["rbc2"] = norm_quake(2, state["cb2"])
                            issue_b(3)
                        if m == 7:
                            for cc in range(4):
                                norm_mult(2, state["rbc2"], cc, nc.gpsimd)
                    if g == 1:
                        if m in (0, 1):
                            sq_chunk(3, 2 * m, nc.vector)
                            sq_chunk(3, 2 * m + 1, nc.vector)
                        if m == 4:
                            state["cb3"] = norm_pre(3, bsq_tiles[3])
                        if m == 5:
                            state["rbc3"] = norm_quake(3, state["cb3"])
                        if m == 6:
                            for cc in range(4):
                                norm_mult(3, state["rbc3"], cc, nc.gpsimd)

            # --- assembly: ship [S | exp(pos)] rows; host takes the lns
            nc.vector.reduce_sum(cat_s[:, 0:NM], acc_s[:], axis=AX.X)
            nc.scalar.dma_start(out_d.ap(), cat_s[:])

    nc.compile()
    return nc


def get_nc():
    if "nc" not in _CACHE:
        _CACHE["nc"] = _build_nc()
    return _CACHE["nc"]


def make_in_maps(A: np.ndarray, B: np.ndarray) -> list[dict]:
    import ml_dtypes

    A = np.asarray(A, dtype=np.float32)
    B = np.asarray(B, dtype=np.float32)
    # view-major D-major matrices: X[d, v*N + n] = X_in[n, v, d]
    At = np.ascontiguousarray(A.transpose(2, 1, 0).reshape(D, M))
    Bt = np.ascontiguousarray(B.transpose(2, 1, 0).reshape(D, M))
    i128 = np.eye(P, dtype=np.float32).astype(ml_dtypes.bfloat16)
    mneg = np.zeros((P, KC, P), dtype=np.float32)
    mneg[:, 0] = np.eye(P) * -16.0
    mneg = mneg.astype(ml_dtypes.float8_e4m3)
    mpos = np.zeros((P, KC, P), dtype=np.float32)
    mpos[:, 0] = np.eye(P) * 16.0
    mpos = mpos.astype(ml_dtypes.float8_e4m3)
    in_maps = []
    for k in range(NCORES):
        atk = At[:, k * ROWS : (k + 1) * ROWS]
        # (D, ROWS) -> (P, KC, ROWS): partition p holds d = kc*128 + p
        at_k = np.ascontiguousarray(
            atk.reshape(KC, P, ROWS).transpose(1, 0, 2)).astype(
                ml_dtypes.float8_e4m3)
        # (P, NM, D): partition p holds rows t*128 + p
        arow_k = np.ascontiguousarray(
            atk.T.reshape(NM, P, D).transpose(1, 0, 2)).astype(
                ml_dtypes.bfloat16)
        # rotate columns so local col j holds global col (j + 1024k) % 8192
        # (P, NG, KC, GW): partition p holds d = kc*128 + p, group-major
        btroll = np.roll(Bt, -ROWS * k, axis=1)
        bt0_k = np.ascontiguousarray(
            btroll[:, :GW].reshape(KC, P, 4, 512).transpose(1, 2, 0, 3)
        ).astype(ml_dtypes.bfloat16)
        btr_k = np.ascontiguousarray(
            btroll[:, GW:].reshape(KC, P, NG - 1, GW).transpose(1, 2, 0, 3)
        ).astype(ml_dtypes.bfloat16)
        in_maps.append({"at": at_k, "arow": arow_k, "bt0": bt0_k,
                        "btr": btr_k, "i128": i128, "mneg": mneg,
                        "mpos": mpos})
    return in_maps


def kernel(A: np.ndarray, B: np.ndarray) -> np.ndarray:
    from concourse.bass_utils import run_bass_kernel_spmd

    in_maps = make_in_maps(A, B)
    nc = get_nc()
    trace = bool(int(os.environ.get("KERNEL_TRACE", "0")))
    res = run_bass_kernel_spmd(
        nc, in_maps, core_ids=list(range(NCORES)), trace=trace)
    total = 0.0
    for r in res.results:
        st = r["stats"].astype(np.float64)
        total += float(np.sum(np.log(st[:, 0:NM]) - np.log(st[:, NM:])))
    if res.exec_time_ns is not None:
        print(f"[kernel] exec_time_ns={res.exec_time_ns}")
        _CACHE["exec_time_ns"] = res.exec_time_ns
    _CACHE["last_results"] = res
    return np.float32(total / M)


# revision 40
# speedup vs baseline: 1.0866x; 1.0866x over previous
"""Contrastive loss (CLIP-style, 2 views) on 8 Trainium2 NeuronCores.

Math: with Af/Bf the L2-normalized (V*N, D) view-major matrices,
  loss = mean_i [ logsumexp_{j != i}(Af@Bf.T / T)[i, :] - (Af@Bf.T)[i, p(i)]/T ]
where p(i) = (i + N) mod (V*N) is the other-view partner of row i.

Sharding: rows of Af are split across 8 cores (1024 rows each); every core
gets the full B (D-major) with its columns rotated by 1024*k so the
diagonal of core k's slab lands at *static* local columns (row-chunk m ->
cols [128m, 128m+128) of column-group 0) and the partner diagonal at the
same offset of column-group 2.  SPMD program identical on all cores.

v2 design (from trace analysis of the 133 us baseline):
- Inputs ship compressed: at fp8e4 (matmul lhsT directly), arow/bt bf16.
  DMA drops 10 MB -> ~5 MB per core, prologue halves.
- ACT runs ONLY: a dummy ln (table preload), the ln/exp rsqrt chains for
  A-row norms + B group-0 column norms (all in the otherwise-idle
  prologue, 2 table loads total), then 24 of the 32 exp tiles.
- 8 exp tiles (odd-m of groups 1 and 3) run on the DVE via the Schraudolph
  int-trick: i32(x*A + B) bit-cast is exp(x) to ~1.8% sawtooth with a
  C constant tuned for zero-mean log error; a reduce pass makes row sums.
- B groups 1-3 column rsqrt: ones-matmul (fp8 DoubleRow) -> PSUM row ->
  SBUF -> DRAM bounce -> compact (128,16) -> Quake rsqrt w/ 1 Newton step
  on the GPSIMD (idle engine; 0.17% max err) -> DRAM -> partition-
  broadcast multiply.  DVE only does the row copy + final multiplies.
- B squares: group 0+1 on DVE (latency critical), 2+3 on GPSIMD.
- All norm matmuls in fp8 DoubleRow (ones fp8), halving their PE cost.
"""

import os

import numpy as np

N = 4096
V = 2
D = 256
M = V * N            # 8192 rows/cols of the logits matrix
TEMP = 0.07
NCORES = 8
ROWS = M // NCORES   # 1024 rows per core
P = 128              # partitions
NM = ROWS // P       # 8 row-chunks per core
GW = 2048            # column-group width (one B normalize unit)
NG = M // GW         # 4 column groups
PSW = 2048           # PSUM tile width (half of PSUM -> 2-deep rotation)
KC = D // P          # 2 contraction chunks
NEG = -1.0e9         # additive mask for the diagonal
MAGIC = 0x5F3759DF   # Quake rsqrt seed
SB_A = 12102203.161561485          # 2^23 / ln 2
SB_B = 1065353216.0 - 480000.0     # 127*2^23 - C, C tuned for 0-mean log err

# which (g, m) exp tiles run on the DVE instead of ACT
def _is_dve_tile(g, m):
    return (g in (1, 3) and m in (1, 5)) or (g == 2 and m in (2, 6))

_CACHE: dict = {}


def _build_nc():
    import concourse.bacc as bacc
    import concourse.bass as bass
    import concourse.mybir as mybir
    import concourse.tile as tile

    f32 = mybir.dt.float32
    i32 = mybir.dt.int32
    bf16 = mybir.dt.bfloat16
    fp8 = mybir.dt.float8e4
    AX = mybir.AxisListType
    OP = mybir.AluOpType
    AF = mybir.ActivationFunctionType
    DR = mybir.MatmulPerfMode.DoubleRow

    nc = bacc.Bacc("TRN2", target_bir_lowering=False, debug=False,
                   num_devices=NCORES)

    at_d = nc.dram_tensor("at", (P, KC, ROWS), fp8, kind="ExternalInput")
    arow_d = nc.dram_tensor("arow", (P, NM, D), bf16, kind="ExternalInput")
    bt0_d = nc.dram_tensor("bt0", (P, 4, KC, 512), bf16, kind="ExternalInput")
    btr_d = nc.dram_tensor("btr", (P, NG - 1, KC, GW), bf16,
                           kind="ExternalInput")
    i128_d = nc.dram_tensor("i128", (P, P), bf16, kind="ExternalInput")
    mneg_d = nc.dram_tensor("mneg", (P, KC, P), fp8, kind="ExternalInput")
    mpos_d = nc.dram_tensor("mpos", (P, KC, P), fp8, kind="ExternalInput")
    out_d = nc.dram_tensor("stats", (P, 2 * NM), f32, kind="ExternalOutput")

    with tile.TileContext(nc) as tc:
        with (
            tc.tile_pool(name="big", bufs=1) as big,
            tc.tile_pool(name="work", bufs=2) as work,
            tc.tile_pool(name="dram", bufs=2,
                         space=bass.MemorySpace.DRAM) as dr,
            tc.tile_pool(name="psum", bufs=2, space=bass.MemorySpace.PSUM) as pp,
        ):
            # --- persistent SBUF tensors -------------------------------
            at_b = big.tile((P, KC, ROWS), fp8)     # A slab (matmul lhsT)
            arow_s = big.tile((P, NM, D), bf16)     # A slab, row-major
            bt_b = big.tile((P, KC, M), fp8)        # normalized B (rhs)
            i128_s = big.tile((P, P), bf16)
            mneg_s = big.tile((P, KC, P), fp8)      # -16 I (diag mask mm)
            mpos_s = big.tile((P, KC, P), fp8)      # +16 I
            ones_b = big.tile((P, KC, P), fp8)      # ones (norm mm)
            ssa_s = big.tile((P, NM), f32)          # sum(a^2) per slab row
            lssa_s = big.tile((P, NM), f32)         # ln of it
            sca_s = big.tile((P, NM), f32)          # 1/(|a|*T) exp scales
            sca2_s = big.tile((P, NM), f32)         # * SB_A, for DVE tiles
            acc_s = big.tile((P, NM, NG), f32)      # exp row-sums per tile
            cat_s = big.tile((P, 2 * NM), f32)      # [S | exp(pos)] per row
            lns0_s = big.tile((P, GW), f32)         # ln of g0 col sumsq
            inv0_s = big.tile((P, GW), bf16)        # g0 col rsqrt
            blnt_s = big.tile((P, 1), f32)          # ln(1/T) bias for sca

            # --- Quake rsqrt, 1 Newton step (max rel err 0.175%) -------
            # shift on DVE (tensor_scalar unsupported on Pool); arithmetic
            # as tensor_tensor on GPSIMD against memset const tiles
            QSH = (16, P)
            magic_s = big.tile(QSH, i32)
            c15_s = big.tile(QSH, f32)
            cnh_s = big.tile(QSH, f32)
            onei_s = big.tile(QSH, i32)

            def quake_rsqrt(out_ap, s_ap):
                ii = work.tile(QSH, i32, tag="qk_i", bufs=3)
                t1 = work.tile(QSH, f32, tag="qk_t", bufs=3)
                nc.vector.tensor_scalar(ii[:], s_ap.bitcast(i32), 1,
                                        None, OP.logical_shift_right)
                nc.gpsimd.tensor_tensor(ii[:], magic_s[:], ii[:],
                                        OP.subtract)
                yb = ii[:].bitcast(f32)
                # y = yb * (1.5 - 0.5*s*yb^2)
                nc.gpsimd.tensor_mul(t1[:], yb, yb)
                nc.gpsimd.tensor_mul(t1[:], t1[:], s_ap)
                nc.gpsimd.tensor_mul(t1[:], t1[:], cnh_s[:])
                nc.gpsimd.tensor_tensor(t1[:], t1[:], c15_s[:], OP.add)
                nc.gpsimd.tensor_mul(out_ap, yb, t1[:])

            # --- table preload + DMA issue order -----------------------
            # pre-place the combined ln+exp table (set 6 in act_info.json)
            # so the insert_act_table_loads fixpoint sees every ln/exp
            # covered and emits nothing -> zero mid-kernel table switches
            nc.scalar.add_instruction(
                mybir.InstLoadActFuncSet(
                    name=nc.get_next_instruction_name(),
                    ins=[], outs=[], act_func_set_id=6))
            nc.scalar.dma_start(arow_s[:], arow_d.ap())
            nc.scalar.dma_start(at_b[:], at_d.ap())
            btf_tiles = []
            for _g in range(NG):
                btf = work.tile((P, KC, GW), bf16, tag="btf", bufs=4)
                btf_tiles.append(btf)

            for c in range(GW // 512):
                csl = slice(c * 512, (c + 1) * 512)
                nc.sync.dma_start(btf_tiles[0][:, :, csl], bt0_d.ap()[:, c])
            nc.sync.dma_start(mneg_s[:], mneg_d.ap())
            nc.sync.dma_start(mpos_s[:], mpos_d.ap())
            nc.sync.dma_start(i128_s[:], i128_d.ap())
            def issue_b(g):
                nc.sync.dma_start(btf_tiles[g][:, :, :], btr_d.ap()[:, g - 1])

            issue_b(1)


            nc.vector.memset(ones_b[:], 1.0)
            nc.vector.memset(blnt_s[:], float(np.log(1.0 / TEMP)))
            nc.gpsimd.memset(magic_s[:], MAGIC)
            nc.gpsimd.memset(onei_s[:], 1)
            nc.gpsimd.memset(c15_s[:], 1.5)
            nc.gpsimd.memset(cnh_s[:], -0.5)

            # --- group 0 normalization: fully chunk-pipelined ----------
            # ones-matmul broadcasts the column sumsq to all partitions;
            # rsqrt = exp(-0.5*ln) per 512-chunk on the idle ACT; chunks
            # 0-1 squared/normalized on DVE, 2-3 on GPSIMD
            btf0 = btf_tiles[0]
            ssb0 = pp.tile((P, PSW), f32, tag="ps", bufs=2)
            bsq0 = work.tile((P, KC, GW), fp8, tag="bsq0", bufs=1)
            for c in range(GW // 512):
                csl = slice(c * 512, (c + 1) * 512)
                if c < 2:
                    nc.scalar.activation(bsq0[:, :, csl], btf0[:, :, csl],
                                         AF.Square)
                else:
                    nc.vector.tensor_mul(bsq0[:, :, csl], btf0[:, :, csl],
                                         btf0[:, :, csl])
                nc.tensor.matmul(ssb0[:, csl], ones_b[:], bsq0[:, :, csl],
                                 start=True, stop=True, perf_mode=DR)
                nc.scalar.activation(lns0_s[:, csl], ssb0[:, csl], AF.Ln)
                nc.scalar.activation(inv0_s[:, csl], lns0_s[:, csl],
                                     AF.Exp, scale=-0.5)
            # A scales on ACT: sca = exp(-0.5*ln(ssa) + ln(1/T))
            for m in range(NM):
                asq = work.tile((P, D), bf16, tag="asq", bufs=2)
                nc.vector.scalar_tensor_tensor(
                    asq[:], arow_s[:, m, :], 0.0, arow_s[:, m, :],
                    OP.bypass, OP.mult,
                    accum_out=ssa_s[:, m : m + 1])
            nc.scalar.activation(lssa_s[:], ssa_s[:], AF.Ln)
            nc.scalar.activation(sca_s[:], lssa_s[:], AF.Exp,
                                 scale=-0.5, bias=blnt_s[:])
            nc.vector.tensor_scalar_mul(sca2_s[:], sca_s[:], SB_A)
            for c in range(GW // 512):
                csl = slice(c * 512, (c + 1) * 512)
                eng0 = nc.vector if c < 2 else nc.gpsimd
                for kc in range(KC):
                    eng0.tensor_mul(bt_b[:, kc, csl], btf0[:, kc, csl],
                                    inv0_s[:, csl])

            # --- B groups 1-3 norm chain pieces ------------------------
            bsq_tiles = {}

            def sq_chunk(g, c, eng0):
                if g not in bsq_tiles:
                    bsqn = work.tile((P, KC, GW), fp8, tag="bsq", bufs=2)
                    bsq_tiles[g] = bsqn
                bsq = bsq_tiles[g]
                csl = slice(c * 512, (c + 1) * 512)
                eng0.tensor_mul(bsq[:, :, csl], btf_tiles[g][:, :, csl],
                                btf_tiles[g][:, :, csl])
                return bsq

            def norm_pre(g, bsq):
                # row-0 sumsq via fp8 DR ones-matmul (chunked so the
                # borrowed PSUM slot frees fast), bf16 row copy, DRAM
                # bounce to a compact (16,128) tile
                ssb = pp.tile((P, PSW), f32, tag="ps", bufs=2)
                brow = work.tile((P, GW), bf16, tag="brow", bufs=2)
                for c in range(GW // 512):
                    csl = slice(c * 512, (c + 1) * 512)
                    nc.tensor.matmul(ssb[:, csl], ones_b[:], bsq[:, :, csl],
                                     start=True, stop=True, perf_mode=DR)
                    nc.vector.tensor_copy(brow[0:1, csl], ssb[0:1, csl])
                dsb = dr.tile((GW,), bf16, tag="dsB", bufs=3)
                nc.sync.dma_start(dsb[:], brow[0:1, :])
                compb = work.tile((16, P), bf16, tag="compb", bufs=3)
                nc.sync.dma_start(
                    compb[:], dsb[:].rearrange("(p c) -> p c", p=16))
                return compb

            def norm_quake(g, compb):
                compf = work.tile((16, P), f32, tag="compf", bufs=3)
                nc.gpsimd.tensor_copy(compf[:], compb[:])
                invb = work.tile((16, P), bf16, tag="invb", bufs=3)
                quake_rsqrt(invb[:], compf[:])
                drb = dr.tile((GW,), bf16, tag="drB", bufs=3)
                nc.sync.dma_start(
                    drb[:].rearrange("(p c) -> p c", p=16), invb[:])
                rbc = work.tile((P, GW), bf16, tag="rbc", bufs=2)
                for c in range(GW // 512):
                    csl = slice(c * 512, (c + 1) * 512)
                    nc.sync.dma_start(
                        rbc[:, csl],
                        drb[:][c * 512 : (c + 1) * 512].partition_broadcast(P))
                return rbc

            def norm_mult(g, rbc, c, eng0):
                base_g = g * GW
                csl = slice(c * 512, (c + 1) * 512)
                osl = slice(base_g + c * 512, base_g + (c + 1) * 512)
                for kc in range(KC):
                    eng0.tensor_mul(bt_b[:, kc, osl],
                                    btf_tiles[g][:, kc, csl], rbc[:, csl])

            for c in range(GW // 512):
                sq_chunk(1, c, nc.vector)

            # --- phase 1: logits + exp row-sums ------------------------
            state = {}
            for g in range(NG):
                base = g * GW
                for m in range(NM):
                    lg = pp.tile((P, PSW), f32, tag="ps", bufs=2)
                    msl = slice(m * P, (m + 1) * P)
                    for c in range(PSW // 512):
                        csl = slice(c * 512, (c + 1) * 512)
                        bsl = slice(base + c * 512, base + (c + 1) * 512)
                        masked = g == 0 and c == (m * P) // 512
                        nc.tensor.matmul(
                            lg[:, csl],
                            at_b[:, :, m * P : (m + 1) * P],
                            bt_b[:, :, bsl],
                            start=True, stop=not masked, perf_mode=DR)
                        if masked:
                            # accumulate -256 on the diagonal -> exp == 0
                            nc.tensor.matmul(
                                lg[:, msl], mneg_s[:], mpos_s[:],
                                start=False, stop=True, perf_mode=DR)
                    if _is_dve_tile(g, m):
                        # Schraudolph: exp(x) ~= bitcast(i32(x*A + B))
                        eint = work.tile((P, PSW), i32, tag="eint", bufs=3)
                        nc.vector.tensor_scalar(
                            eint[:], lg[:], sca2_s[:, m : m + 1], SB_B,
                            OP.mult, OP.add)
                        nc.vector.reduce_sum(
                            acc_s[:, m, g : g + 1], eint[:].bitcast(f32),
                            axis=AX.X)
                        if g == 2:
                            pscr = work.tile((P, P), bf16, tag="pscr",
                                             bufs=3)
                            nc.vector.scalar_tensor_tensor(
                                pscr[:], eint[:, msl].bitcast(f32), 0.0,
                                i128_s[:], OP.bypass, OP.mult,
                                accum_out=cat_s[:, NM + m : NM + m + 1])
                    else:
                        esc = work.tile((P, PSW), bf16, tag="esc", bufs=4)
                        nc.scalar.activation(
                            esc[:], lg[:], AF.Exp,
                            scale=sca_s[:, m : m + 1],
                            accum_out=acc_s[:, m, g : g + 1])
                        if g == 2:
                            # partner (positive): ln(exp diag) on host
                            pscr = work.tile((P, P), bf16, tag="pscr",
                                             bufs=3)
                            nc.vector.scalar_tensor_tensor(
                                pscr[:], esc[:, msl], 0.0, i128_s[:],
                                OP.bypass, OP.mult,
                                accum_out=cat_s[:, NM + m : NM + m + 1])
                    # norm chains: GPSIMD is the dedicated chain engine
                    # (quake + all normalize mults) so bounce latency
                    # never head-of-line blocks a sweep engine; ssb
                    # matmuls enter the PE stream early, when squares
                    # are already done
                    if g == 0:
                        if m == 0:
                            state["cb1"] = norm_pre(1, bsq_tiles[1])
                        if m == 1:
                            state["rbc1"] = norm_quake(1, state["cb1"])
                        if m == 2:
                            issue_b(2)
                            for cc in range(4):
                                norm_mult(1, state["rbc1"], cc, nc.gpsimd)
                        if m in (3, 4):
                            sq_chunk(2, 2 * (m - 3), nc.gpsimd)
                            sq_chunk(2, 2 * (m - 3) + 1, nc.gpsimd)
                        if m == 5:
                            state["cb2"] = norm_pre(2, bsq_tiles[2])
                        if m == 6:
                            state["rbc2"] = norm_quake(2, state["cb2"])
                            issue_b(3)
                        if m == 7:
                            for cc in range(4):
                                norm_mult(2, state["rbc2"], cc, nc.gpsimd)
                    if g == 1:
                        if m in (0, 1):
                            sq_chunk(3, 2 * m, nc.gpsimd)
                            sq_chunk(3, 2 * m + 1, nc.gpsimd)
                        if m == 4:
                            state["cb3"] = norm_pre(3, bsq_tiles[3])
                        if m == 5:
                            state["rbc3"] = norm_quake(3, state["cb3"])
                        if m == 6:
                            for cc in range(4):
                                norm_mult(3, state["rbc3"], cc, nc.gpsimd)

            # --- assembly: ship [S | exp(pos)] rows; host takes the lns
            nc.vector.reduce_sum(cat_s[:, 0:NM], acc_s[:], axis=AX.X)
            nc.scalar.dma_start(out_d.ap(), cat_s[:])

    nc.compile()
    return nc


def get_nc():
    if "nc" not in _CACHE:
        _CACHE["nc"] = _build_nc()
    return _CACHE["nc"]


def make_in_maps(A: np.ndarray, B: np.ndarray) -> list[dict]:
    import ml_dtypes

    A = np.asarray(A, dtype=np.float32)
    B = np.asarray(B, dtype=np.float32)
    # view-major D-major matrices: X[d, v*N + n] = X_in[n, v, d]
    At = np.ascontiguousarray(A.transpose(2, 1, 0).reshape(D, M))
    Bt = np.ascontiguousarray(B.transpose(2, 1, 0).reshape(D, M))
    i128 = np.eye(P, dtype=np.float32).astype(ml_dtypes.bfloat16)
    mneg = np.zeros((P, KC, P), dtype=np.float32)
    mneg[:, 0] = np.eye(P) * -16.0
    mneg = mneg.astype(ml_dtypes.float8_e4m3)
    mpos = np.zeros((P, KC, P), dtype=np.float32)
    mpos[:, 0] = np.eye(P) * 16.0
    mpos = mpos.astype(ml_dtypes.float8_e4m3)
    in_maps = []
    for k in range(NCORES):
        atk = At[:, k * ROWS : (k + 1) * ROWS]
        # (D, ROWS) -> (P, KC, ROWS): partition p holds d = kc*128 + p
        at_k = np.ascontiguousarray(
            atk.reshape(KC, P, ROWS).transpose(1, 0, 2)).astype(
                ml_dtypes.float8_e4m3)
        # (P, NM, D): partition p holds rows t*128 + p
        arow_k = np.ascontiguousarray(
            atk.T.reshape(NM, P, D).transpose(1, 0, 2)).astype(
                ml_dtypes.bfloat16)
        # rotate columns so local col j holds global col (j + 1024k) % 8192
        # (P, NG, KC, GW): partition p holds d = kc*128 + p, group-major
        btroll = np.roll(Bt, -ROWS * k, axis=1)
        bt0_k = np.ascontiguousarray(
            btroll[:, :GW].reshape(KC, P, 4, 512).transpose(1, 2, 0, 3)
        ).astype(ml_dtypes.bfloat16)
        btr_k = np.ascontiguousarray(
            btroll[:, GW:].reshape(KC, P, NG - 1, GW).transpose(1, 2, 0, 3)
        ).astype(ml_dtypes.bfloat16)
        in_maps.append({"at": at_k, "arow": arow_k, "bt0": bt0_k,
                        "btr": btr_k, "i128": i128, "mneg": mneg,
                        "mpos": mpos})
    return in_maps


def kernel(A: np.ndarray, B: np.ndarray) -> np.ndarray:
    from concourse.bass_utils import run_bass_kernel_spmd

    in_maps = make_in_maps(A, B)
    nc = get_nc()
    trace = bool(int(os.environ.get("KERNEL_TRACE", "0")))
    res = run_bass_kernel_spmd(
        nc, in_maps, core_ids=list(range(NCORES)), trace=trace)
    total = 0.0
    for r in res.results:
        st = r["stats"].astype(np.float64)
        total += float(np.sum(np.log(st[:, 0:NM]) - np.log(st[:, NM:])))
    if res.exec_time_ns is not None:
        print(f"[kernel] exec_time_ns={res.exec_time_ns}")
        _CACHE["exec_time_ns"] = res.exec_time_ns
    _CACHE["last_results"] = res
    return np.float32(total / M)


# revision 41
# speedup vs baseline: 1.2526x; 1.1528x over previous
"""Contrastive loss (CLIP-style, 2 views) on 8 Trainium2 NeuronCores.

Math: with Af/Bf the L2-normalized (V*N, D) view-major matrices,
  loss = mean_i [ logsumexp_{j != i}(Af@Bf.T / T)[i, :] - (Af@Bf.T)[i, p(i)]/T ]
where p(i) = (i + N) mod (V*N) is the other-view partner of row i.

Sharding: rows of Af are split across 8 cores (1024 rows each); every core
gets the full B (D-major) with its columns rotated by 1024*k so the
diagonal of core k's slab lands at *static* local columns (row-chunk m ->
cols [128m, 128m+128) of column-group 0) and the partner diagonal at the
same offset of column-group 2.  SPMD program identical on all cores.

v2 design (from trace analysis of the 133 us baseline):
- Inputs ship compressed: at fp8e4 (matmul lhsT directly), arow/bt bf16.
  DMA drops 10 MB -> ~5 MB per core, prologue halves.
- ACT runs ONLY: a dummy ln (table preload), the ln/exp rsqrt chains for
  A-row norms + B group-0 column norms (all in the otherwise-idle
  prologue, 2 table loads total), then 24 of the 32 exp tiles.
- 8 exp tiles (odd-m of groups 1 and 3) run on the DVE via the Schraudolph
  int-trick: i32(x*A + B) bit-cast is exp(x) to ~1.8% sawtooth with a
  C constant tuned for zero-mean log error; a reduce pass makes row sums.
- B groups 1-3 column rsqrt: ones-matmul (fp8 DoubleRow) -> PSUM row ->
  SBUF -> DRAM bounce -> compact (128,16) -> Quake rsqrt w/ 1 Newton step
  on the GPSIMD (idle engine; 0.17% max err) -> DRAM -> partition-
  broadcast multiply.  DVE only does the row copy + final multiplies.
- B squares: group 0+1 on DVE (latency critical), 2+3 on GPSIMD.
- All norm matmuls in fp8 DoubleRow (ones fp8), halving their PE cost.
"""

import os

import numpy as np

N = 4096
V = 2
D = 256
M = V * N            # 8192 rows/cols of the logits matrix
TEMP = 0.07
NCORES = 8
ROWS = M // NCORES   # 1024 rows per core
P = 128              # partitions
NM = ROWS // P       # 8 row-chunks per core
GW = 2048            # column-group width (one B normalize unit)
NG = M // GW         # 4 column groups
PSW = 2048           # PSUM tile width (half of PSUM -> 2-deep rotation)
KC = D // P          # 2 contraction chunks
NEG = -1.0e9         # additive mask for the diagonal
MAGIC = 0x5F3759DF   # Quake rsqrt seed
SB_A = 12102203.161561485          # 2^23 / ln 2
SB_B = 1065353216.0 - 480000.0     # 127*2^23 - C, C tuned for 0-mean log err

# which (g, m) exp tiles run on the DVE instead of ACT
def _is_dve_tile(g, m):
    return (g in (1, 3) and m in (1, 5)) or (g == 2 and m in (2, 6))

_CACHE: dict = {}


def _build_nc():
    import concourse.bacc as bacc
    import concourse.bass as bass
    import concourse.mybir as mybir
    import concourse.tile as tile

    f32 = mybir.dt.float32
    i32 = mybir.dt.int32
    bf16 = mybir.dt.bfloat16
    fp8 = mybir.dt.float8e4
    AX = mybir.AxisListType
    OP = mybir.AluOpType
    AF = mybir.ActivationFunctionType
    DR = mybir.MatmulPerfMode.DoubleRow

    nc = bacc.Bacc("TRN2", target_bir_lowering=False, debug=False,
                   num_devices=NCORES)

    at_d = nc.dram_tensor("at", (P, KC, ROWS), fp8, kind="ExternalInput")
    arow_d = nc.dram_tensor("arow", (P, NM, D), bf16, kind="ExternalInput")
    bt0_d = nc.dram_tensor("bt0", (P, 4, KC, 512), bf16, kind="ExternalInput")
    btr_d = nc.dram_tensor("btr", (P, NG - 1, KC, GW), bf16,
                           kind="ExternalInput")
    i128_d = nc.dram_tensor("i128", (P, P), bf16, kind="ExternalInput")
    mneg_d = nc.dram_tensor("mneg", (P, KC, P), fp8, kind="ExternalInput")
    mpos_d = nc.dram_tensor("mpos", (P, KC, P), fp8, kind="ExternalInput")
    out_d = nc.dram_tensor("stats", (P, 2 * NM), f32, kind="ExternalOutput")

    with tile.TileContext(nc) as tc:
        with (
            tc.tile_pool(name="big", bufs=1) as big,
            tc.tile_pool(name="work", bufs=2) as work,
            tc.tile_pool(name="dram", bufs=2,
                         space=bass.MemorySpace.DRAM) as dr,
            tc.tile_pool(name="psum", bufs=2, space=bass.MemorySpace.PSUM) as pp,
        ):
            # --- persistent SBUF tensors -------------------------------
            at_b = big.tile((P, KC, ROWS), fp8)     # A slab (matmul lhsT)
            arow_s = big.tile((P, NM, D), bf16)     # A slab, row-major
            bt_b = big.tile((P, KC, M), fp8)        # normalized B (rhs)
            i128_s = big.tile((P, P), bf16)
            mneg_s = big.tile((P, KC, P), fp8)      # -16 I (diag mask mm)
            mpos_s = big.tile((P, KC, P), fp8)      # +16 I
            ones_b = big.tile((P, KC, P), fp8)      # ones (norm mm)
            ssa_s = big.tile((P, NM), f32)          # sum(a^2) per slab row
            lssa_s = big.tile((P, NM), f32)         # ln of it
            sca_s = big.tile((P, NM), f32)          # 1/(|a|*T) exp scales
            sca2_s = big.tile((P, NM), f32)         # * SB_A, for DVE tiles
            acc_s = big.tile((P, NM, NG), f32)      # exp row-sums per tile
            cat_s = big.tile((P, 2 * NM), f32)      # [S | exp(pos)] per row
            lns0_s = big.tile((P, GW), f32)         # ln of g0 col sumsq
            inv0_s = big.tile((P, GW), bf16)        # g0 col rsqrt
            blnt_s = big.tile((P, 1), f32)          # ln(1/T) bias for sca

            # --- Quake rsqrt, 1 Newton step (max rel err 0.175%) -------
            # shift on DVE (tensor_scalar unsupported on Pool); arithmetic
            # as tensor_tensor on GPSIMD against memset const tiles
            QSH = (16, P)
            magic_s = big.tile(QSH, i32)
            c15_s = big.tile(QSH, f32)
            cnh_s = big.tile(QSH, f32)
            onei_s = big.tile(QSH, i32)

            def quake_rsqrt(out_ap, s_ap):
                ii = work.tile(QSH, i32, tag="qk_i", bufs=3)
                t1 = work.tile(QSH, f32, tag="qk_t", bufs=3)
                nc.vector.tensor_scalar(ii[:], s_ap.bitcast(i32), 1,
                                        None, OP.logical_shift_right)
                nc.gpsimd.tensor_tensor(ii[:], magic_s[:], ii[:],
                                        OP.subtract)
                yb = ii[:].bitcast(f32)
                # y = yb * (1.5 - 0.5*s*yb^2)
                nc.gpsimd.tensor_mul(t1[:], yb, yb)
                nc.gpsimd.tensor_mul(t1[:], t1[:], s_ap)
                nc.gpsimd.tensor_mul(t1[:], t1[:], cnh_s[:])
                nc.gpsimd.tensor_tensor(t1[:], t1[:], c15_s[:], OP.add)
                nc.gpsimd.tensor_mul(out_ap, yb, t1[:])

            # --- table preload + DMA issue order -----------------------
            # pre-place the combined ln+exp table (set 6 in act_info.json)
            # so the insert_act_table_loads fixpoint sees every ln/exp
            # covered and emits nothing -> zero mid-kernel table switches
            nc.scalar.add_instruction(
                mybir.InstLoadActFuncSet(
                    name=nc.get_next_instruction_name(),
                    ins=[], outs=[], act_func_set_id=6))
            nc.scalar.dma_start(arow_s[:], arow_d.ap())
            nc.scalar.dma_start(at_b[:], at_d.ap())
            btf_tiles = []
            for _g in range(NG):
                btf = work.tile((P, KC, GW), bf16, tag="btf", bufs=4)
                btf_tiles.append(btf)

            for c in range(GW // 512):
                csl = slice(c * 512, (c + 1) * 512)
                nc.sync.dma_start(btf_tiles[0][:, :, csl], bt0_d.ap()[:, c])
            nc.sync.dma_start(mneg_s[:], mneg_d.ap())
            nc.sync.dma_start(mpos_s[:], mpos_d.ap())
            nc.sync.dma_start(i128_s[:], i128_d.ap())
            def issue_b(g):
                nc.sync.dma_start(btf_tiles[g][:, :, :], btr_d.ap()[:, g - 1])

            issue_b(1)


            nc.vector.memset(ones_b[:], 1.0)
            nc.vector.memset(blnt_s[:], float(np.log(1.0 / TEMP)))
            nc.gpsimd.memset(magic_s[:], MAGIC)
            nc.gpsimd.memset(onei_s[:], 1)
            nc.gpsimd.memset(c15_s[:], 1.5)
            nc.gpsimd.memset(cnh_s[:], -0.5)

            # --- group 0 normalization: fully chunk-pipelined ----------
            # ones-matmul broadcasts the column sumsq to all partitions;
            # rsqrt = exp(-0.5*ln) per 512-chunk on the idle ACT; chunks
            # 0-1 squared/normalized on DVE, 2-3 on GPSIMD
            btf0 = btf_tiles[0]
            ssb0 = pp.tile((P, PSW), f32, tag="ps", bufs=2)
            bsq0 = work.tile((P, KC, GW), fp8, tag="bsq0", bufs=1)
            for c in range(GW // 512):
                csl = slice(c * 512, (c + 1) * 512)
                if c < 2:
                    nc.scalar.activation(bsq0[:, :, csl], btf0[:, :, csl],
                                         AF.Square)
                else:
                    nc.vector.tensor_mul(bsq0[:, :, csl], btf0[:, :, csl],
                                         btf0[:, :, csl])
                nc.tensor.matmul(ssb0[:, csl], ones_b[:], bsq0[:, :, csl],
                                 start=True, stop=True, perf_mode=DR)
                nc.scalar.activation(lns0_s[:, csl], ssb0[:, csl], AF.Ln)
                nc.scalar.activation(inv0_s[:, csl], lns0_s[:, csl],
                                     AF.Exp, scale=-0.5)
            # A scales on ACT: sca = exp(-0.5*ln(ssa) + ln(1/T))
            for m in range(NM):
                asq = work.tile((P, D), bf16, tag="asq", bufs=2)
                nc.vector.scalar_tensor_tensor(
                    asq[:], arow_s[:, m, :], 0.0, arow_s[:, m, :],
                    OP.bypass, OP.mult,
                    accum_out=ssa_s[:, m : m + 1])
            nc.scalar.activation(lssa_s[:], ssa_s[:], AF.Ln)
            nc.scalar.activation(sca_s[:], lssa_s[:], AF.Exp,
                                 scale=-0.5, bias=blnt_s[:])
            nc.vector.tensor_scalar_mul(sca2_s[:], sca_s[:], SB_A)
            for c in range(GW // 512):
                csl = slice(c * 512, (c + 1) * 512)
                eng0 = nc.vector if c < 2 else nc.gpsimd
                for kc in range(KC):
                    eng0.tensor_mul(bt_b[:, kc, csl], btf0[:, kc, csl],
                                    inv0_s[:, csl])

            # --- B groups 1-3 norm chain pieces ------------------------
            bsq_tiles = {}

            def sq_chunk(g, c, eng0):
                if g not in bsq_tiles:
                    bsqn = work.tile((P, KC, GW), fp8, tag="bsq", bufs=2)
                    bsq_tiles[g] = bsqn
                bsq = bsq_tiles[g]
                csl = slice(c * 512, (c + 1) * 512)
                eng0.tensor_mul(bsq[:, :, csl], btf_tiles[g][:, :, csl],
                                btf_tiles[g][:, :, csl])
                return bsq

            def norm_pre(g, bsq):
                # row-0 sumsq via fp8 DR ones-matmul (chunked so the
                # borrowed PSUM slot frees fast), bf16 row copy, DRAM
                # bounce to a compact (16,128) tile
                ssb = pp.tile((P, PSW), f32, tag="ps", bufs=2)
                brow = work.tile((P, GW), bf16, tag="brow", bufs=2)
                for c in range(GW // 512):
                    csl = slice(c * 512, (c + 1) * 512)
                    nc.tensor.matmul(ssb[:, csl], ones_b[:], bsq[:, :, csl],
                                     start=True, stop=True, perf_mode=DR)
                    nc.vector.tensor_copy(brow[0:1, csl], ssb[0:1, csl])
                dsb = dr.tile((GW,), bf16, tag="dsB", bufs=3)
                nc.sync.dma_start(dsb[:], brow[0:1, :])
                compb = work.tile((16, P), bf16, tag="compb", bufs=3)
                nc.sync.dma_start(
                    compb[:], dsb[:].rearrange("(p c) -> p c", p=16))
                return compb

            def norm_quake(g, compb):
                compf = work.tile((16, P), f32, tag="compf", bufs=3)
                nc.gpsimd.tensor_copy(compf[:], compb[:])
                invb = work.tile((16, P), bf16, tag="invb", bufs=3)
                quake_rsqrt(invb[:], compf[:])
                drb = dr.tile((GW,), bf16, tag="drB", bufs=3)
                nc.sync.dma_start(
                    drb[:].rearrange("(p c) -> p c", p=16), invb[:])
                rbc = work.tile((P, GW), bf16, tag="rbc", bufs=2)
                for c in range(GW // 512):
                    csl = slice(c * 512, (c + 1) * 512)
                    nc.sync.dma_start(
                        rbc[:, csl],
                        drb[:][c * 512 : (c + 1) * 512].partition_broadcast(P))
                return rbc

            def norm_mult(g, rbc, c, eng0):
                base_g = g * GW
                csl = slice(c * 512, (c + 1) * 512)
                osl = slice(base_g + c * 512, base_g + (c + 1) * 512)
                for kc in range(KC):
                    eng0.tensor_mul(bt_b[:, kc, osl],
                                    btf_tiles[g][:, kc, csl], rbc[:, csl])

            for c in range(GW // 512):
                sq_chunk(1, c, nc.vector)

            # --- phase 1: logits + exp row-sums ------------------------
            state = {}
            for g in range(NG):
                base = g * GW
                for m in range(NM):
                    lg = pp.tile((P, PSW), f32, tag="ps", bufs=2)
                    msl = slice(m * P, (m + 1) * P)
                    for c in range(PSW // 512):
                        csl = slice(c * 512, (c + 1) * 512)
                        bsl = slice(base + c * 512, base + (c + 1) * 512)
                        masked = g == 0 and c == (m * P) // 512
                        nc.tensor.matmul(
                            lg[:, csl],
                            at_b[:, :, m * P : (m + 1) * P],
                            bt_b[:, :, bsl],
                            start=True, stop=not masked, perf_mode=DR)
                        if masked:
                            # accumulate -256 on the diagonal -> exp == 0
                            nc.tensor.matmul(
                                lg[:, msl], mneg_s[:], mpos_s[:],
                                start=False, stop=True, perf_mode=DR)
                    if _is_dve_tile(g, m):
                        # Schraudolph: exp(x) ~= bitcast(i32(x*A + B))
                        eint = work.tile((P, PSW), i32, tag="eint", bufs=3)
                        nc.vector.tensor_scalar(
                            eint[:], lg[:], sca2_s[:, m : m + 1], SB_B,
                            OP.mult, OP.add)
                        nc.vector.reduce_sum(
                            acc_s[:, m, g : g + 1], eint[:].bitcast(f32),
                            axis=AX.X)
                        if g == 2:
                            pscr = work.tile((P, P), bf16, tag="pscr",
                                             bufs=3)
                            nc.vector.scalar_tensor_tensor(
                                pscr[:], eint[:, msl].bitcast(f32), 0.0,
                                i128_s[:], OP.bypass, OP.mult,
                                accum_out=cat_s[:, NM + m : NM + m + 1])
                    else:
                        esc = work.tile((P, PSW), bf16, tag="esc", bufs=4)
                        nc.scalar.activation(
                            esc[:], lg[:], AF.Exp,
                            scale=sca_s[:, m : m + 1],
                            accum_out=acc_s[:, m, g : g + 1])
                        if g == 2:
                            # partner (positive): ln(exp diag) on host
                            pscr = work.tile((P, P), bf16, tag="pscr",
                                             bufs=3)
                            nc.vector.scalar_tensor_tensor(
                                pscr[:], esc[:, msl], 0.0, i128_s[:],
                                OP.bypass, OP.mult,
                                accum_out=cat_s[:, NM + m : NM + m + 1])
                    # norm chains: GPSIMD is the dedicated chain engine
                    # (quake + all normalize mults) so bounce latency
                    # never head-of-line blocks a sweep engine; ssb
                    # matmuls enter the PE stream early, when squares
                    # are already done
                    if g == 0:
                        if m == 1:
                            state["cb1"] = norm_pre(1, bsq_tiles[1])
                        if m == 2:
                            state["rbc1"] = norm_quake(1, state["cb1"])
                        if m == 3:
                            issue_b(2)
                            for cc in range(4):
                                norm_mult(1, state["rbc1"], cc, nc.gpsimd)
                        if m in (3, 4):
                            sq_chunk(2, 2 * (m - 3), nc.vector)
                            sq_chunk(2, 2 * (m - 3) + 1, nc.vector)
                        if m == 5:
                            state["cb2"] = norm_pre(2, bsq_tiles[2])
                        if m == 6:
                            state["rbc2"] = norm_quake(2, state["cb2"])
                            issue_b(3)
                        if m == 7:
                            for cc in range(4):
                                norm_mult(2, state["rbc2"], cc, nc.gpsimd)
                    if g == 1:
                        if m in (0, 1):
                            sq_chunk(3, 2 * m, nc.vector)
                            sq_chunk(3, 2 * m + 1, nc.vector)
                        if m == 4:
                            state["cb3"] = norm_pre(3, bsq_tiles[3])
                        if m == 5:
                            state["rbc3"] = norm_quake(3, state["cb3"])
                        if m == 6:
                            for cc in range(4):
                                norm_mult(3, state["rbc3"], cc, nc.gpsimd)

            # --- assembly: ship [S | exp(pos)] rows; host takes the lns
            nc.vector.reduce_sum(cat_s[:, 0:NM], acc_s[:], axis=AX.X)
            nc.scalar.dma_start(out_d.ap(), cat_s[:])

    nc.compile()
    return nc


def get_nc():
    if "nc" not in _CACHE:
        _CACHE["nc"] = _build_nc()
    return _CACHE["nc"]


def make_in_maps(A: np.ndarray, B: np.ndarray) -> list[dict]:
    import ml_dtypes

    A = np.asarray(A, dtype=np.float32)
    B = np.asarray(B, dtype=np.float32)
    # view-major D-major matrices: X[d, v*N + n] = X_in[n, v, d]
    At = np.ascontiguousarray(A.transpose(2, 1, 0).reshape(D, M))
    Bt = np.ascontiguousarray(B.transpose(2, 1, 0).reshape(D, M))
    i128 = np.eye(P, dtype=np.float32).astype(ml_dtypes.bfloat16)
    mneg = np.zeros((P, KC, P), dtype=np.float32)
    mneg[:, 0] = np.eye(P) * -16.0
    mneg = mneg.astype(ml_dtypes.float8_e4m3)
    mpos = np.zeros((P, KC, P), dtype=np.float32)
    mpos[:, 0] = np.eye(P) * 16.0
    mpos = mpos.astype(ml_dtypes.float8_e4m3)
    in_maps = []
    for k in range(NCORES):
        atk = At[:, k * ROWS : (k + 1) * ROWS]
        # (D, ROWS) -> (P, KC, ROWS): partition p holds d = kc*128 + p
        at_k = np.ascontiguousarray(
            atk.reshape(KC, P, ROWS).transpose(1, 0, 2)).astype(
                ml_dtypes.float8_e4m3)
        # (P, NM, D): partition p holds rows t*128 + p
        arow_k = np.ascontiguousarray(
            atk.T.reshape(NM, P, D).transpose(1, 0, 2)).astype(
                ml_dtypes.bfloat16)
        # rotate columns so local col j holds global col (j + 1024k) % 8192
        # (P, NG, KC, GW): partition p holds d = kc*128 + p, group-major
        btroll = np.roll(Bt, -ROWS * k, axis=1)
        bt0_k = np.ascontiguousarray(
            btroll[:, :GW].reshape(KC, P, 4, 512).transpose(1, 2, 0, 3)
        ).astype(ml_dtypes.bfloat16)
        btr_k = np.ascontiguousarray(
            btroll[:, GW:].reshape(KC, P, NG - 1, GW).transpose(1, 2, 0, 3)
        ).astype(ml_dtypes.bfloat16)
        in_maps.append({"at": at_k, "arow": arow_k, "bt0": bt0_k,
                        "btr": btr_k, "i128": i128, "mneg": mneg,
                        "mpos": mpos})
    return in_maps


def kernel(A: np.ndarray, B: np.ndarray) -> np.ndarray:
    from concourse.bass_utils import run_bass_kernel_spmd

    in_maps = make_in_maps(A, B)
    nc = get_nc()
    trace = bool(int(os.environ.get("KERNEL_TRACE", "0")))
    res = run_bass_kernel_spmd(
        nc, in_maps, core_ids=list(range(NCORES)), trace=trace)
    total = 0.0
    for r in res.results:
        st = r["stats"].astype(np.float64)
        total += float(np.sum(np.log(st[:, 0:NM]) - np.log(st[:, NM:])))
    if res.exec_time_ns is not None:
        print(f"[kernel] exec_time_ns={res.exec_time_ns}")
        _CACHE["exec_time_ns"] = res.exec_time_ns
    _CACHE["last_results"] = res
    return np.float32(total / M)
